# revision 40
# baseline (speedup 1.0000x reference)
"""EM-routing (matrix capsules) Trainium2 kernel.

Problem: nn_BaseCaps — N=512, K=288 (in-caps*kernel), C=32 (out-caps), P=16 (pose).
Sharding: out_caps C=32 -> 4 per core across 8 cores (tensor parallel, fully
local math: all reductions are over K/P, and the e-step softmax over N is
per-(k,c) which stays on-core).

Per-core algorithm (layout: n on 128 partitions, free axes (k, c, p)):
  for i in 0..R-1:
    m-step: S0 = sum_k rr, S1 = sum_k rr*v, S2 = sum_k rr*v^2
            means = S1/S0, var = S2/S0 - means^2, std = sqrt(var)
            act = sigmoid(lambda_i * (beta_a - 16*beta_v - S0 * sum_p log(std+eps)))
    e-step (i < R-1):
            pe[n,k,c]  = sum_p (v*is - means*is)^2   (is = 1/std)
            t[n,k,c]   = exp(-0.5*pe + ln(act+eps) - sum_p log(std+eps))
            D[k,c]     = sum_n t  (TensorE ones-matmul, accumulated over n-slabs)
            rr_next    = t / D   (softmax over axis 0 without max-shift;
                                  equal to jax.nn.softmax(zz, axis=0))
"""

import sys
from contextlib import ExitStack

import numpy as np

if "/opt/trn_rl_repo" not in sys.path:
    sys.path.insert(0, "/opt/trn_rl_repo")

import os

INVD_P0 = os.environ.get("KK_INVD_P0", "1") == "1"

# problem dims (hardcoded per harness contract)
N, K, C, P = 512, 288, 32, 16
NCORES = 8
CL = C // NCORES  # out-caps per core
NP = 128          # partitions per slab
EPS = 1e-7


def emit_core_kernel(ctx, tc, outs, ins, R, dims=None):
    """Emit the per-core program. outs=(out_act[N,CL], out_means[N,CL,P]),
    ins=(votes[N,K,CL,P], beta_a[CL], beta_v[CL])."""
    import concourse.mybir as mybir

    n, k, cl, p = dims if dims is not None else (N, K, CL, P)
    nt = n // NP
    f32 = mybir.dt.float32
    AF = mybir.ActivationFunctionType
    AX = mybir.AxisListType
    OP = mybir.AluOpType

    nc = tc.nc
    out_act, out_means = outs
    votes, beta_a, beta_v = ins

    kcl = k * cl
    MMCOL = 384  # f32 cols per PSUM-bank matmul target
    dchunks = [(j, min(MMCOL, kcl - j)) for j in range(0, kcl, MMCOL)]

    # ---- pools ----
    pv = ctx.enter_context(tc.tile_pool(name="pv", bufs=2))       # votes slab
    pc = ctx.enter_context(tc.tile_pool(name="pc", bufs=2))       # [128,k,p/2] scratch
    pscr = ctx.enter_context(tc.tile_pool(name="pscr", bufs=2))   # [128,k] scratch
    pt = ctx.enter_context(tc.tile_pool(name="pt", bufs=1))       # persistent t slabs
    pr = ctx.enter_context(tc.tile_pool(name="pr", bufs=2))       # rr
    pid = ctx.enter_context(tc.tile_pool(name="pid", bufs=1))     # invD_b
    psml = ctx.enter_context(tc.tile_pool(name="psml", bufs=2))   # small stats
    pone = ctx.enter_context(tc.tile_pool(name="pone", bufs=1))   # constants
    ppsD = ctx.enter_context(tc.tile_pool(name="ppsD", bufs=1, space="PSUM"))
    ppsB = ctx.enter_context(tc.tile_pool(name="ppsB", bufs=1, space="PSUM"))

    # ---- constants ----
    zero_c = pone.tile([NP, 1], f32, tag="zero_c", name="zero_c")
    nc.vector.memset(zero_c[:], 0.0)
    eps_c = pone.tile([NP, 1], f32, tag="eps_c", name="eps_c")
    nc.vector.memset(eps_c[:], EPS)
    nc.const_aps.aps[(f32, 0.0)] = zero_c[:]
    nc.const_aps.aps[(f32, EPS)] = eps_c[:]

    ones_col = pone.tile([NP, 1], f32, tag="ones_col")
    nc.vector.memset(ones_col[:], 1.0)
    ones_row = pone.tile([1, NP], f32, tag="ones_row")
    nc.vector.memset(ones_row[:], 1.0)

    ba_b = pone.tile([NP, cl], f32, tag="ba_b")
    nc.sync.dma_start(out=ba_b[:], in_=beta_a.unsqueeze(0).broadcast_to([NP, cl]))
    bv_b = pone.tile([NP, cl], f32, tag="bv_b")
    nc.sync.dma_start(out=bv_b[:], in_=beta_v.unsqueeze(0).broadcast_to([NP, cl]))
    # ba16bv = beta_a - P*beta_v
    bv16 = pone.tile([NP, cl], f32, tag="bv16")
    nc.scalar.mul(bv16[:], bv_b[:], float(p))
    ba16bv = pone.tile([NP, cl], f32, tag="ba16bv")
    nc.vector.tensor_sub(ba16bv[:], ba_b[:], bv16[:])

    # persistent t slabs (softmax numerators), [128, k, cl] each
    t_sl = [pt.tile([NP, k, cl], f32, tag=f"tsl{s}", name=f"tsl{s}") for s in range(nt)]

    invD_b = None  # [128, k, cl] broadcast of 1/D, refreshed per e-step

    S0_CONST0 = float(k) / float(C)  # sum_k (1/C) for iteration 0

    v_cur_idx, v_cur = -1, None  # slab kept resident across phase boundary

    for i in range(R):
        lambd = 0.01 * (1.0 - 0.95 ** i)
        is_last = i == R - 1

        if not is_last:
            d_ps = [ppsD.tile([1, sz], f32, tag=f"d{j}", name=f"d{j}") for j, (_, sz) in enumerate(dchunks)]

        def emit_pe(s, si, v_sl, is_, btn, lnw):
            ph = p // 2
            for c in range(cl):
                pe_c = psml.tile([NP, k], f32, tag="pe_c", name="pe_c")
                for h in range(2):
                    ucb = pc.tile([NP, k, ph], f32, tag="cbuf", name=f"ucb{c}{h}")
                    for pp in range(h * ph, (h + 1) * ph):
                        nc.scalar.activation(
                            out=ucb[:, :, pp - h * ph], in_=v_sl[:, :, c, pp],
                            func=AF.Square,
                            bias=btn[:, c, pp:pp + 1], scale=is_[:, c, pp:pp + 1])
                    if h == 0:
                        nc.vector.tensor_reduce(
                            out=pe_c[:], in_=ucb[:], axis=AX.X, op=OP.add)
                    else:
                        pe_h = psml.tile([NP, k], f32, tag="pe_h", name="pe_h")
                        nc.vector.tensor_reduce(
                            out=pe_h[:], in_=ucb[:], axis=AX.X, op=OP.add)
                        nc.vector.tensor_add(pe_c[:], pe_c[:], pe_h[:])
                nc.scalar.activation(
                    out=t_sl[s][:, :, c], in_=pe_c[:], func=AF.Exp,
                    bias=lnw[:, c:c + 1], scale=-0.5)
            # partial D accumulation: D[k,c] += sum_n t
            t_flat = t_sl[s][:].rearrange("q a b -> q (a b)")
            for j, (j0, sz) in enumerate(dchunks):
                nc.tensor.matmul(
                    out=d_ps[j][:], lhsT=ones_col[:], rhs=t_flat[:, j0:j0 + sz],
                    start=(si == 0), stop=(si == nt - 1))

        pending = None  # pe work lagged by one slab to overlap with next S-block
        order = list(range(nt)) if i % 2 == 0 else list(reversed(range(nt)))
        for si, s in enumerate(order):
            ns = s * NP
            if s == v_cur_idx and i > 0:
                v_sl = v_cur  # still resident from previous phase
            else:
                v_sl = pv.tile([NP, k, cl, p], f32, tag="vslab", name="vslab")
                nc.sync.dma_start(out=v_sl[:], in_=votes[ns:ns + NP])
            v_cur_idx, v_cur = s, v_sl

            S1 = psml.tile([NP, cl, p], f32, tag="S1")
            S2 = psml.tile([NP, cl, p], f32, tag="S2")

            ph = p // 2

            def se_red(src_slice, accum_slot, func=AF.Copy):
                """k-sum on ScalarE: accum_out = sum_k func(src); out -> junk."""
                junk = pscr.tile([NP, k], f32, tag="junk", name="junk")
                nc.scalar.activation(out=junk[:], in_=src_slice, func=func,
                                     scale=1.0, accum_out=accum_slot)

            if i == 0:
                # uniform rr = 1/C: raw sums over k
                nc.vector.tensor_reduce(
                    out=S1[:], in_=v_sl[:].transpose([0, 2, 3, 1]), axis=AX.X, op=OP.add)
                # S2 = sum_k v^2: half the (c,h) groups fused on SE
                # (Square+accum per pose), half as SE-square + VE group reduce
                n_s20 = int(os.environ.get("KK_NS20", "4"))
                for c in range(cl):
                    for h in range(2):
                        sl = slice(h * ph, (h + 1) * ph)
                        if (c * 2 + h) % 2 < (n_s20 / 4.0):
                            for pp in range(h * ph, (h + 1) * ph):
                                se_red(v_sl[:, :, c, pp], S2[:, c, pp:pp + 1],
                                       func=AF.Square)
                        else:
                            sq_c = pc.tile([NP, k, ph], f32, tag="cbuf", name="sqc")
                            nc.scalar.square(sq_c[:], v_sl[:, :, c, sl])
                            nc.vector.tensor_reduce(
                                out=S2[:, c, sl],
                                in_=sq_c[:].transpose([0, 2, 1]), axis=AX.X, op=OP.add)
            else:
                rr = pr.tile([NP, k, cl], f32, tag="rr")
                nc.vector.tensor_mul(rr[:], t_sl[s][:], invD_b[:])
                S0 = psml.tile([NP, cl], f32, tag="S0")
                nc.vector.tensor_reduce(
                    out=S0[:], in_=rr[:].transpose([0, 2, 1]), axis=AX.X, op=OP.add)
                # products on VE; k-reduces split between SE activation-accum
                # and VE group reduces, h-pairs interleaved so VE's in-place
                # second multiply overlaps SE's reads of the sibling group
                n_s1_se = int(os.environ.get("KK_NS1", "8"))
                n_s2_se = int(os.environ.get("KK_NS2", "0"))
                for c in range(cl):
                    prods = []
                    for h in range(2):
                        sl = slice(h * ph, (h + 1) * ph)
                        prod_c = pc.tile([NP, k, ph], f32, tag="cbuf", name="prodc")
                        vs = v_sl[:, :, c, sl]
                        rr_bc = rr[:, :, c].unsqueeze(-1).broadcast_to([NP, k, ph])
                        nc.vector.tensor_mul(prod_c[:], vs, rr_bc)
                        prods.append((prod_c, vs, sl, h))
                    for prod_c, vs, sl, h in prods:
                        if (c * 2 + h) < n_s1_se:
                            for jj in range(ph):
                                se_red(prod_c[:, :, jj],
                                       S1[:, c, h * ph + jj:h * ph + jj + 1])
                        else:
                            nc.vector.tensor_reduce(
                                out=S1[:, c, sl],
                                in_=prod_c[:].transpose([0, 2, 1]), axis=AX.X, op=OP.add)
                    for prod_c, vs, sl, h in prods:
                        nc.vector.tensor_mul(prod_c[:], prod_c[:], vs)
                        if (c * 2 + h) < n_s2_se:
                            for jj in range(ph):
                                se_red(prod_c[:, :, jj],
                                       S2[:, c, h * ph + jj:h * ph + jj + 1])
                        else:
                            nc.vector.tensor_reduce(
                                out=S2[:, c, sl],
                                in_=prod_c[:].transpose([0, 2, 1]), axis=AX.X, op=OP.add)

            # ---- m-step finalize ----
            means = psml.tile([NP, cl, p], f32, tag="means")
            ex2 = psml.tile([NP, cl, p], f32, tag="ex2")
            if i == 0:
                nc.scalar.mul(means[:], S1[:], 1.0 / float(k))
                nc.scalar.mul(ex2[:], S2[:], 1.0 / float(k))
            else:
                invS0 = psml.tile([NP, cl], f32, tag="invS0")
                nc.vector.reciprocal(invS0[:], S0[:])
                is0b = invS0[:].unsqueeze(-1).broadcast_to([NP, cl, p])
                nc.vector.tensor_mul(means[:], S1[:], is0b)
                nc.vector.tensor_mul(ex2[:], S2[:], is0b)
            msq = psml.tile([NP, cl, p], f32, tag="msq")
            nc.scalar.square(msq[:], means[:])
            var = psml.tile([NP, cl, p], f32, tag="var")
            nc.vector.tensor_sub(var[:], ex2[:], msq[:])
            std = psml.tile([NP, cl, p], f32, tag="std")
            nc.scalar.sqrt(std[:], var[:])
            logstd = psml.tile([NP, cl, p], f32, tag="logstd")
            nc.scalar.activation(logstd[:], std[:], AF.Ln, bias=EPS, scale=1.0)
            sumlog = psml.tile([NP, cl], f32, tag="sumlog")
            nc.vector.tensor_reduce(out=sumlog[:], in_=logstd[:], axis=AX.X, op=OP.add)

            cterm = psml.tile([NP, cl], f32, tag="cterm")
            if i == 0:
                nc.scalar.mul(cterm[:], sumlog[:], S0_CONST0)
            else:
                nc.vector.tensor_mul(cterm[:], S0[:], sumlog[:])
            actin = psml.tile([NP, cl], f32, tag="actin")
            nc.vector.tensor_sub(actin[:], ba16bv[:], cterm[:])
            act = psml.tile([NP, cl], f32, tag="act")
            nc.scalar.activation(act[:], actin[:], AF.Sigmoid, bias=0.0, scale=float(lambd))

            if is_last:
                nc.sync.dma_start(out=out_act[ns:ns + NP], in_=act[:])
                nc.sync.dma_start(out=out_means[ns:ns + NP], in_=means[:])
                continue

            # ---- e-step ----
            is_ = psml.tile([NP, cl, p], f32, tag="is_")
            nc.vector.reciprocal(is_[:], std[:])
            mis = psml.tile([NP, cl, p], f32, tag="mis")
            nc.vector.tensor_mul(mis[:], means[:], is_[:])
            btn = psml.tile([NP, cl, p], f32, tag="btn")
            nc.scalar.mul(btn[:], mis[:], -1.0)
            lnact = psml.tile([NP, cl], f32, tag="lnact")
            nc.scalar.activation(lnact[:], act[:], AF.Ln, bias=EPS, scale=1.0)
            lnw = psml.tile([NP, cl], f32, tag="lnw")
            nc.vector.tensor_sub(lnw[:], lnact[:], sumlog[:])

            emit_pe(s, si, v_sl, is_, btn, lnw)

        if is_last:
            continue

        # ---- finalize D -> invD broadcast across partitions ----
        # reciprocal lands in partition 0 of invD_b, then a [1,128]-ones
        # matmul replicates it to all partitions (overwriting row 0 equally)
        invD_b = pid.tile([NP, k, cl], f32, tag="invD_b")
        invDb_flat = invD_b[:].rearrange("q a b -> q (a b)")
        if INVD_P0:
            row = invDb_flat[0:1, :]
        else:
            invD_row = pone.tile([1, kcl], f32, tag="invD_row", name="invD_row")
            row = invD_row[:]
        for j, (j0, sz) in enumerate(dchunks):
            nc.vector.reciprocal(row[:, j0:j0 + sz], d_ps[j][:])
        for j, (j0, sz) in enumerate(dchunks):
            b_ps = ppsB.tile([NP, MMCOL], f32, tag=f"b{j}", name=f"b{j}")
            nc.tensor.matmul(
                out=b_ps[:, :sz], lhsT=ones_row[:], rhs=row[:, j0:j0 + sz],
                start=True, stop=True)
            nc.vector.tensor_copy(out=invDb_flat[:, j0:j0 + sz], in_=b_ps[:, :sz])


def build_program(R, dims=None):
    import concourse.bacc as bacc
    import concourse.mybir as mybir
    import concourse.tile as tile

    n, k, cl, p = dims if dims is not None else (N, K, CL, P)
    f32 = mybir.dt.float32

    nc = bacc.Bacc("TRN2", target_bir_lowering=False, debug=False,
                   enable_asserts=False, num_devices=NCORES)
    votes = nc.dram_tensor("votes", [n, k, cl, p], f32, kind="ExternalInput")
    beta_a = nc.dram_tensor("beta_a", [cl], f32, kind="ExternalInput")
    beta_v = nc.dram_tensor("beta_v", [cl], f32, kind="ExternalInput")
    out_act = nc.dram_tensor("out_act", [n, cl], f32, kind="ExternalOutput")
    out_means = nc.dram_tensor("out_means", [n, cl, p], f32, kind="ExternalOutput")

    with tile.TileContext(nc) as tc:
        with ExitStack() as ctx:
            emit_core_kernel(
                ctx, tc,
                (out_act.ap(), out_means.ap()),
                (votes.ap(), beta_a.ap(), beta_v.ap()),
                R, dims=dims)
    nc.compile()
    return nc


_CACHE = {}


def _get_program(R):
    if R not in _CACHE:
        _CACHE[R] = build_program(R)
    return _CACHE[R]


_RUNNER = {}


def _get_runner(R):
    """Build the sharded PJRT executable once per R (mirrors
    bass2jax.run_bass_via_pjrt's multi-core path, cached across calls)."""
    if R in _RUNNER:
        return _RUNNER[R]
    import jax
    from jax.sharding import Mesh, PartitionSpec
    from jax.experimental.shard_map import shard_map
    import concourse.mybir as mybir
    from concourse import bass2jax

    nc = _get_program(R)
    bass2jax.install_neuronx_cc_hook()

    partition_name = nc.partition_id_tensor.name if nc.partition_id_tensor else None
    in_names, out_names, out_avals, zero_shapes = [], [], [], []
    for alloc in nc.m.functions[0].allocations:
        if not isinstance(alloc, mybir.MemoryLocationSet):
            continue
        name = alloc.memorylocations[0].name
        if alloc.kind == "ExternalInput":
            if name != partition_name:
                in_names.append(name)
        elif alloc.kind == "ExternalOutput":
            out_names.append(name)
            shape = tuple(alloc.tensor_shape)
            dtype = mybir.dt.np(alloc.dtype)
            out_avals.append(jax.core.ShapedArray(shape, dtype))
            zero_shapes.append((shape, dtype))
    n_params = len(in_names)
    n_outs = len(out_avals)
    all_in_names = list(in_names) + list(out_names)
    if partition_name is not None:
        all_in_names.append(partition_name)
    donate = tuple(range(n_params, n_params + n_outs))

    def _body(*args):
        operands = list(args)
        if partition_name is not None:
            operands.append(bass2jax.partition_id_tensor())
        outs = bass2jax._bass_exec_p.bind(
            *operands,
            out_avals=tuple(out_avals),
            in_names=tuple(all_in_names),
            out_names=tuple(out_names),
            lowering_input_output_aliases=(),
            sim_require_finite=True,
            sim_require_nnan=True,
            nc=nc,
        )
        return tuple(outs)

    devices = jax.devices()[:NCORES]
    mesh = Mesh(np.asarray(devices), ("core",))
    in_specs = (PartitionSpec("core"),) * (n_params + n_outs)
    out_specs = (PartitionSpec("core"),) * n_outs
    sharded = jax.jit(
        shard_map(_body, mesh=mesh, in_specs=in_specs, out_specs=out_specs,
                  check_rep=False),
        donate_argnums=donate, keep_unused=True)

    _RUNNER[R] = (sharded, in_names, out_names, out_avals, zero_shapes)
    return _RUNNER[R]


def kernel(**inputs):
    votes = np.asarray(inputs["votes"], dtype=np.float32)
    beta_a = np.asarray(inputs["beta_a"], dtype=np.float32).reshape(-1)
    beta_v = np.asarray(inputs["beta_v"], dtype=np.float32).reshape(-1)
    R = int(inputs.get("routings", 3))
    assert votes.shape == (N, K, C, P)

    in_maps = []
    for j in range(NCORES):
        cs = j * CL
        in_maps.append({
            "votes": np.ascontiguousarray(votes[:, :, cs:cs + CL, :]),
            "beta_a": np.ascontiguousarray(beta_a[cs:cs + CL]),
            "beta_v": np.ascontiguousarray(beta_v[cs:cs + CL]),
        })

    sharded, in_names, out_names, out_avals, zero_shapes = _get_runner(R)
    concat_in = [
        np.concatenate([np.asarray(m[name]) for m in in_maps], axis=0)
        for name in in_names
    ]
    concat_zeros = [
        np.zeros((NCORES * s[0], *s[1:]), d) for s, d in zero_shapes
    ]
    out_arrs = sharded(*concat_in, *concat_zeros)
    res = {
        name: np.asarray(out_arrs[i]).reshape(NCORES, *out_avals[i].shape)
        for i, name in enumerate(out_names)
    }
    out_act = np.concatenate(
        [res["out_act"][c].reshape(N, 1, CL, 1) for c in range(NCORES)], axis=2)
    out_means = np.concatenate(
        [res["out_means"][c].reshape(N, 1, CL, P) for c in range(NCORES)], axis=2)
    return out_act, out_means


# revision 47
# speedup vs baseline: 1.5541x; 1.5541x over previous
"""EM-routing (matrix capsules) Trainium2 kernel.

Problem: nn_BaseCaps — N=512, K=288 (in-caps*kernel), C=32 (out-caps), P=16 (pose).
Sharding: out_caps C=32 -> 4 per core across 8 cores (tensor parallel, fully
local math: all reductions are over K/P, and the e-step softmax over N is
per-(k,c) which stays on-core).

Per-core algorithm (layout: n on 128 partitions, free axes (k, c, p)):
  for i in 0..R-1:
    m-step: S0 = sum_k rr, S1 = sum_k rr*v, S2 = sum_k rr*v^2
            means = S1/S0, var = S2/S0 - means^2, std = sqrt(var)
            act = sigmoid(lambda_i * (beta_a - 16*beta_v - S0 * sum_p log(std+eps)))
    e-step (i < R-1):
            pe[n,k,c]  = sum_p (v*is - means*is)^2   (is = 1/std)
            t[n,k,c]   = exp(-0.5*pe + ln(act+eps) - sum_p log(std+eps))
            D[k,c]     = sum_n t  (TensorE ones-matmul, accumulated over n-slabs)
            rr_next    = t / D   (softmax over axis 0 without max-shift;
                                  equal to jax.nn.softmax(zz, axis=0))
"""

import sys
from contextlib import ExitStack

import numpy as np

if "/opt/trn_rl_repo" not in sys.path:
    sys.path.insert(0, "/opt/trn_rl_repo")

import os

INVD_P0 = os.environ.get("KK_INVD_P0", "1") == "1"

# problem dims (hardcoded per harness contract)
N, K, C, P = 512, 288, 32, 16
NCORES = 8
CL = C // NCORES  # out-caps per core
NP = 128          # partitions per slab
EPS = 1e-7


def emit_core_kernel(ctx, tc, outs, ins, R, dims=None):
    """Emit the per-core program. outs=(out_act[N,CL], out_means[N,CL,P]),
    ins=(votes[N,K,CL,P], beta_a[CL], beta_v[CL])."""
    import concourse.mybir as mybir

    n, k, cl, p = dims if dims is not None else (N, K, CL, P)
    nt = n // NP
    f32 = mybir.dt.float32
    AF = mybir.ActivationFunctionType
    AX = mybir.AxisListType
    OP = mybir.AluOpType

    nc = tc.nc
    out_act, out_means = outs
    votes, beta_a, beta_v = ins

    kcl = k * cl
    MMCOL = 384  # f32 cols per PSUM-bank matmul target
    dchunks = [(j, min(MMCOL, kcl - j)) for j in range(0, kcl, MMCOL)]

    # ---- pools ----
    pv = ctx.enter_context(tc.tile_pool(name="pv", bufs=2))       # votes slab
    pc = ctx.enter_context(tc.tile_pool(name="pc", bufs=2))       # [128,k,p/2] scratch
    pscr = ctx.enter_context(tc.tile_pool(name="pscr", bufs=2))   # [128,k] scratch
    pt = ctx.enter_context(tc.tile_pool(name="pt", bufs=1))       # persistent t slabs
    pr = ctx.enter_context(tc.tile_pool(
        name="pr", bufs=int(os.environ.get("KK_RRBUFS", "2"))))  # rr
    pid = ctx.enter_context(tc.tile_pool(name="pid", bufs=1))     # invD_b
    psml = ctx.enter_context(tc.tile_pool(
        name="psml", bufs=int(os.environ.get("KK_SMLBUFS", "2"))))  # small stats
    pone = ctx.enter_context(tc.tile_pool(name="pone", bufs=1))   # constants
    ppsD = ctx.enter_context(tc.tile_pool(name="ppsD", bufs=1, space="PSUM"))
    ppsB = ctx.enter_context(tc.tile_pool(name="ppsB", bufs=1, space="PSUM"))

    # ---- constants ----
    zero_c = pone.tile([NP, 1], f32, tag="zero_c", name="zero_c")
    nc.vector.memset(zero_c[:], 0.0)
    eps_c = pone.tile([NP, 1], f32, tag="eps_c", name="eps_c")
    nc.vector.memset(eps_c[:], EPS)
    nc.const_aps.aps[(f32, 0.0)] = zero_c[:]
    nc.const_aps.aps[(f32, EPS)] = eps_c[:]

    ones_col = pone.tile([NP, 1], f32, tag="ones_col")
    nc.vector.memset(ones_col[:], 1.0)
    ones_row = pone.tile([1, NP], f32, tag="ones_row")
    nc.vector.memset(ones_row[:], 1.0)

    ba_b = pone.tile([NP, cl], f32, tag="ba_b")
    nc.sync.dma_start(out=ba_b[:], in_=beta_a.unsqueeze(0).broadcast_to([NP, cl]))
    bv_b = pone.tile([NP, cl], f32, tag="bv_b")
    nc.sync.dma_start(out=bv_b[:], in_=beta_v.unsqueeze(0).broadcast_to([NP, cl]))
    # ba16bv = beta_a - P*beta_v
    bv16 = pone.tile([NP, cl], f32, tag="bv16")
    nc.scalar.mul(bv16[:], bv_b[:], float(p))
    ba16bv = pone.tile([NP, cl], f32, tag="ba16bv")
    nc.vector.tensor_sub(ba16bv[:], ba_b[:], bv16[:])

    # persistent t slabs (softmax numerators), [128, k, cl] each
    t_sl = [pt.tile([NP, k, cl], f32, tag=f"tsl{s}", name=f"tsl{s}") for s in range(nt)]

    invD_b = None  # [128, k, cl] broadcast of 1/D, refreshed per e-step

    S0_CONST0 = float(k) / float(C)  # sum_k (1/C) for iteration 0

    v_cur_idx, v_cur = -1, None  # slab kept resident across phase boundary

    for i in range(R):
        lambd = 0.01 * (1.0 - 0.95 ** i)
        is_last = i == R - 1

        if not is_last:
            d_ps = [ppsD.tile([1, sz], f32, tag=f"d{j}", name=f"d{j}") for j, (_, sz) in enumerate(dchunks)]

        def emit_pe(s, si, v_sl, is_, btn, lnw):
            ph = p // 2
            for c in range(cl):
                pe_c = pscr.tile([NP, k], f32, tag="pe_c", name="pe_c")
                for h in range(2):
                    ucb = pc.tile([NP, k, ph], f32, tag="cbuf", name=f"ucb{c}{h}")
                    for pp in range(h * ph, (h + 1) * ph):
                        nc.scalar.activation(
                            out=ucb[:, :, pp - h * ph], in_=v_sl[:, :, c, pp],
                            func=AF.Square,
                            bias=btn[:, c, pp:pp + 1], scale=is_[:, c, pp:pp + 1])
                    if h == 0:
                        nc.vector.tensor_reduce(
                            out=pe_c[:], in_=ucb[:], axis=AX.X, op=OP.add)
                    else:
                        pe_h = pscr.tile([NP, k], f32, tag="pe_h", name="pe_h")
                        nc.vector.tensor_reduce(
                            out=pe_h[:], in_=ucb[:], axis=AX.X, op=OP.add)
                        nc.vector.tensor_add(pe_c[:], pe_c[:], pe_h[:])
                nc.scalar.activation(
                    out=t_sl[s][:, :, c], in_=pe_c[:], func=AF.Exp,
                    bias=lnw[:, c:c + 1], scale=-0.5)
            # partial D accumulation: D[k,c] += sum_n t
            t_flat = t_sl[s][:].rearrange("q a b -> q (a b)")
            for j, (j0, sz) in enumerate(dchunks):
                nc.tensor.matmul(
                    out=d_ps[j][:], lhsT=ones_col[:], rhs=t_flat[:, j0:j0 + sz],
                    start=(si == 0), stop=(si == nt - 1))

        pending = None  # pe work lagged by one slab to overlap with next S-block
        order = list(range(nt)) if i % 2 == 0 else list(reversed(range(nt)))
        for si, s in enumerate(order):
            ns = s * NP
            if s == v_cur_idx and i > 0:
                v_sl = v_cur  # still resident from previous phase
            else:
                v_sl = pv.tile([NP, k, cl, p], f32, tag="vslab", name="vslab")
                nc.sync.dma_start(out=v_sl[:], in_=votes[ns:ns + NP])
            v_cur_idx, v_cur = s, v_sl

            S1 = psml.tile([NP, cl, p], f32, tag="S1")
            S2 = psml.tile([NP, cl, p], f32, tag="S2")

            ph = p // 2

            def se_red(src_slice, accum_slot, func=AF.Copy):
                """k-sum on ScalarE: accum_out = sum_k func(src); out -> junk."""
                junk = pscr.tile([NP, k], f32, tag="junk", name="junk")
                nc.scalar.activation(out=junk[:], in_=src_slice, func=func,
                                     scale=1.0, accum_out=accum_slot)

            if i == 0:
                # uniform rr = 1/C: raw sums over k
                nc.vector.tensor_reduce(
                    out=S1[:], in_=v_sl[:].transpose([0, 2, 3, 1]), axis=AX.X, op=OP.add)
                # S2 = sum_k v^2: half the (c,h) groups fused on SE
                # (Square+accum per pose), half as SE-square + VE group reduce
                n_s20 = int(os.environ.get("KK_NS20", "4"))
                for c in range(cl):
                    for h in range(2):
                        sl = slice(h * ph, (h + 1) * ph)
                        if (c * 2 + h) % 2 < (n_s20 / 4.0):
                            for pp in range(h * ph, (h + 1) * ph):
                                se_red(v_sl[:, :, c, pp], S2[:, c, pp:pp + 1],
                                       func=AF.Square)
                        else:
                            sq_c = pc.tile([NP, k, ph], f32, tag="cbuf", name="sqc")
                            nc.scalar.square(sq_c[:], v_sl[:, :, c, sl])
                            nc.vector.tensor_reduce(
                                out=S2[:, c, sl],
                                in_=sq_c[:].transpose([0, 2, 1]), axis=AX.X, op=OP.add)
            else:
                rr = pr.tile([NP, k, cl], f32, tag="rr")
                nc.vector.tensor_mul(rr[:], t_sl[s][:], invD_b[:])
                S0 = psml.tile([NP, cl], f32, tag="S0")
                nc.vector.tensor_reduce(
                    out=S0[:], in_=rr[:].transpose([0, 2, 1]), axis=AX.X, op=OP.add)
                # products on VE; k-reduces split between SE activation-accum
                # and VE group reduces, h-pairs interleaved so VE's in-place
                # second multiply overlaps SE's reads of the sibling group.
                # For n_q groups, S2 = sum_k (sqrt(rr)*v)^2 via SE Square-accum
                # on q = v*sqrt(rr) — q is independent of the prod->S1 chain.
                n_s1_se = int(os.environ.get("KK_NS1", "8"))
                n_s2_se = int(os.environ.get("KK_NS2", "0"))
                n_q = int(os.environ.get("KK_NQ", "0"))
                srr = None
                if n_q > 0:
                    srr = pr.tile([NP, k, cl], f32, tag="srr", name="srr", bufs=1)
                    nc.scalar.sqrt(srr[:], rr[:])
                for c in range(cl):
                    prods = []
                    for h in range(2):
                        sl = slice(h * ph, (h + 1) * ph)
                        prod_c = pc.tile([NP, k, ph], f32, tag="cbuf", name="prodc")
                        vs = v_sl[:, :, c, sl]
                        rr_bc = rr[:, :, c].unsqueeze(-1).broadcast_to([NP, k, ph])
                        nc.vector.tensor_mul(prod_c[:], vs, rr_bc)
                        prods.append((prod_c, vs, sl, h))
                    for prod_c, vs, sl, h in prods:
                        if (c * 2 + h) < n_s1_se:
                            for jj in range(ph):
                                se_red(prod_c[:, :, jj],
                                       S1[:, c, h * ph + jj:h * ph + jj + 1])
                        else:
                            nc.vector.tensor_reduce(
                                out=S1[:, c, sl],
                                in_=prod_c[:].transpose([0, 2, 1]), axis=AX.X, op=OP.add)
                    for prod_c, vs, sl, h in prods:
                        if (c * 2 + h) < n_q:
                            q_c = pc.tile([NP, k, ph], f32, tag="cbuf", name="qc")
                            srr_bc = srr[:, :, c].unsqueeze(-1).broadcast_to([NP, k, ph])
                            nc.vector.tensor_mul(q_c[:], vs, srr_bc)
                            for jj in range(ph):
                                se_red(q_c[:, :, jj],
                                       S2[:, c, h * ph + jj:h * ph + jj + 1],
                                       func=AF.Square)
                        else:
                            nc.vector.tensor_mul(prod_c[:], prod_c[:], vs)
                            if (c * 2 + h) < n_s2_se:
                                for jj in range(ph):
                                    se_red(prod_c[:, :, jj],
                                           S2[:, c, h * ph + jj:h * ph + jj + 1])
                            else:
                                nc.vector.tensor_reduce(
                                    out=S2[:, c, sl],
                                    in_=prod_c[:].transpose([0, 2, 1]), axis=AX.X, op=OP.add)

            # ---- m-step finalize ----
            means = psml.tile([NP, cl, p], f32, tag="means")
            ex2 = psml.tile([NP, cl, p], f32, tag="ex2")
            if i == 0:
                nc.scalar.mul(means[:], S1[:], 1.0 / float(k))
                nc.scalar.mul(ex2[:], S2[:], 1.0 / float(k))
            else:
                invS0 = psml.tile([NP, cl], f32, tag="invS0")
                nc.vector.reciprocal(invS0[:], S0[:])
                is0b = invS0[:].unsqueeze(-1).broadcast_to([NP, cl, p])
                nc.vector.tensor_mul(means[:], S1[:], is0b)
                nc.vector.tensor_mul(ex2[:], S2[:], is0b)
            msq = psml.tile([NP, cl, p], f32, tag="msq")
            nc.scalar.square(msq[:], means[:])
            var = psml.tile([NP, cl, p], f32, tag="var")
            nc.vector.tensor_sub(var[:], ex2[:], msq[:])
            std = psml.tile([NP, cl, p], f32, tag="std")
            nc.scalar.sqrt(std[:], var[:])
            logstd = psml.tile([NP, cl, p], f32, tag="logstd")
            nc.scalar.activation(logstd[:], std[:], AF.Ln, bias=EPS, scale=1.0)
            sumlog = psml.tile([NP, cl], f32, tag="sumlog")
            nc.vector.tensor_reduce(out=sumlog[:], in_=logstd[:], axis=AX.X, op=OP.add)

            cterm = psml.tile([NP, cl], f32, tag="cterm")
            if i == 0:
                nc.scalar.mul(cterm[:], sumlog[:], S0_CONST0)
            else:
                nc.vector.tensor_mul(cterm[:], S0[:], sumlog[:])
            actin = psml.tile([NP, cl], f32, tag="actin")
            nc.vector.tensor_sub(actin[:], ba16bv[:], cterm[:])
            act = psml.tile([NP, cl], f32, tag="act")
            nc.scalar.activation(act[:], actin[:], AF.Sigmoid, bias=0.0, scale=float(lambd))

            if is_last:
                nc.sync.dma_start(out=out_act[ns:ns + NP], in_=act[:])
                nc.sync.dma_start(out=out_means[ns:ns + NP], in_=means[:])
                continue

            # ---- e-step ----
            is_ = psml.tile([NP, cl, p], f32, tag="is_")
            nc.vector.reciprocal(is_[:], std[:])
            mis = psml.tile([NP, cl, p], f32, tag="mis")
            nc.vector.tensor_mul(mis[:], means[:], is_[:])
            btn = psml.tile([NP, cl, p], f32, tag="btn")
            nc.scalar.mul(btn[:], mis[:], -1.0)
            lnact = psml.tile([NP, cl], f32, tag="lnact")
            nc.scalar.activation(lnact[:], act[:], AF.Ln, bias=EPS, scale=1.0)
            lnw = psml.tile([NP, cl], f32, tag="lnw")
            nc.vector.tensor_sub(lnw[:], lnact[:], sumlog[:])

            emit_pe(s, si, v_sl, is_, btn, lnw)

        if is_last:
            continue

        # ---- finalize D -> invD broadcast across partitions ----
        # reciprocal lands in partition 0 of invD_b, then a [1,128]-ones
        # matmul replicates it to all partitions (overwriting row 0 equally)
        invD_b = pid.tile([NP, k, cl], f32, tag="invD_b")
        invDb_flat = invD_b[:].rearrange("q a b -> q (a b)")
        if INVD_P0:
            row = invDb_flat[0:1, :]
        else:
            invD_row = pone.tile([1, kcl], f32, tag="invD_row", name="invD_row")
            row = invD_row[:]
        for j, (j0, sz) in enumerate(dchunks):
            nc.vector.reciprocal(row[:, j0:j0 + sz], d_ps[j][:])
        for j, (j0, sz) in enumerate(dchunks):
            b_ps = ppsB.tile([NP, MMCOL], f32, tag=f"b{j}", name=f"b{j}")
            nc.tensor.matmul(
                out=b_ps[:, :sz], lhsT=ones_row[:], rhs=row[:, j0:j0 + sz],
                start=True, stop=True)
            nc.vector.tensor_copy(out=invDb_flat[:, j0:j0 + sz], in_=b_ps[:, :sz])


def build_program(R, dims=None):
    import concourse.bacc as bacc
    import concourse.mybir as mybir
    import concourse.tile as tile

    n, k, cl, p = dims if dims is not None else (N, K, CL, P)
    f32 = mybir.dt.float32

    nc = bacc.Bacc("TRN2", target_bir_lowering=False, debug=False,
                   enable_asserts=False, num_devices=NCORES)
    votes = nc.dram_tensor("votes", [n, k, cl, p], f32, kind="ExternalInput")
    beta_a = nc.dram_tensor("beta_a", [cl], f32, kind="ExternalInput")
    beta_v = nc.dram_tensor("beta_v", [cl], f32, kind="ExternalInput")
    out_act = nc.dram_tensor("out_act", [n, cl], f32, kind="ExternalOutput")
    out_means = nc.dram_tensor("out_means", [n, cl, p], f32, kind="ExternalOutput")

    with tile.TileContext(nc) as tc:
        with ExitStack() as ctx:
            emit_core_kernel(
                ctx, tc,
                (out_act.ap(), out_means.ap()),
                (votes.ap(), beta_a.ap(), beta_v.ap()),
                R, dims=dims)
    nc.compile()
    return nc


_CACHE = {}


def _get_program(R):
    if R not in _CACHE:
        _CACHE[R] = build_program(R)
    return _CACHE[R]


_RUNNER = {}


def _get_runner(R):
    """Build the sharded PJRT executable once per R (mirrors
    bass2jax.run_bass_via_pjrt's multi-core path, cached across calls)."""
    if R in _RUNNER:
        return _RUNNER[R]
    import jax
    from jax.sharding import Mesh, PartitionSpec
    from jax.experimental.shard_map import shard_map
    import concourse.mybir as mybir
    from concourse import bass2jax

    nc = _get_program(R)
    bass2jax.install_neuronx_cc_hook()

    partition_name = nc.partition_id_tensor.name if nc.partition_id_tensor else None
    in_names, out_names, out_avals, zero_shapes = [], [], [], []
    for alloc in nc.m.functions[0].allocations:
        if not isinstance(alloc, mybir.MemoryLocationSet):
            continue
        name = alloc.memorylocations[0].name
        if alloc.kind == "ExternalInput":
            if name != partition_name:
                in_names.append(name)
        elif alloc.kind == "ExternalOutput":
            out_names.append(name)
            shape = tuple(alloc.tensor_shape)
            dtype = mybir.dt.np(alloc.dtype)
            out_avals.append(jax.core.ShapedArray(shape, dtype))
            zero_shapes.append((shape, dtype))
    n_params = len(in_names)
    n_outs = len(out_avals)
    all_in_names = list(in_names) + list(out_names)
    if partition_name is not None:
        all_in_names.append(partition_name)
    donate = tuple(range(n_params, n_params + n_outs))

    def _body(*args):
        operands = list(args)
        if partition_name is not None:
            operands.append(bass2jax.partition_id_tensor())
        outs = bass2jax._bass_exec_p.bind(
            *operands,
            out_avals=tuple(out_avals),
            in_names=tuple(all_in_names),
            out_names=tuple(out_names),
            lowering_input_output_aliases=(),
            sim_require_finite=True,
            sim_require_nnan=True,
            nc=nc,
        )
        return tuple(outs)

    devices = jax.devices()[:NCORES]
    mesh = Mesh(np.asarray(devices), ("core",))
    in_specs = (PartitionSpec("core"),) * (n_params + n_outs)
    out_specs = (PartitionSpec("core"),) * n_outs
    sharded = jax.jit(
        shard_map(_body, mesh=mesh, in_specs=in_specs, out_specs=out_specs,
                  check_rep=False),
        donate_argnums=donate, keep_unused=True)

    _RUNNER[R] = (sharded, in_names, out_names, out_avals, zero_shapes)
    return _RUNNER[R]


def kernel(**inputs):
    votes = np.asarray(inputs["votes"], dtype=np.float32)
    beta_a = np.asarray(inputs["beta_a"], dtype=np.float32).reshape(-1)
    beta_v = np.asarray(inputs["beta_v"], dtype=np.float32).reshape(-1)
    R = int(inputs.get("routings", 3))
    assert votes.shape == (N, K, C, P)

    in_maps = []
    for j in range(NCORES):
        cs = j * CL
        in_maps.append({
            "votes": np.ascontiguousarray(votes[:, :, cs:cs + CL, :]),
            "beta_a": np.ascontiguousarray(beta_a[cs:cs + CL]),
            "beta_v": np.ascontiguousarray(beta_v[cs:cs + CL]),
        })

    sharded, in_names, out_names, out_avals, zero_shapes = _get_runner(R)
    concat_in = [
        np.concatenate([np.asarray(m[name]) for m in in_maps], axis=0)
        for name in in_names
    ]
    concat_zeros = [
        np.zeros((NCORES * s[0], *s[1:]), d) for s, d in zero_shapes
    ]
    out_arrs = sharded(*concat_in, *concat_zeros)
    res = {
        name: np.asarray(out_arrs[i]).reshape(NCORES, *out_avals[i].shape)
        for i, name in enumerate(out_names)
    }
    out_act = np.concatenate(
        [res["out_act"][c].reshape(N, 1, CL, 1) for c in range(NCORES)], axis=2)
    out_means = np.concatenate(
        [res["out_means"][c].reshape(N, 1, CL, P) for c in range(NCORES)], axis=2)
    return out_act, out_means


# revision 53
# speedup vs baseline: 1.6369x; 1.0533x over previous
"""EM-routing (matrix capsules) Trainium2 kernel.

Problem: nn_BaseCaps — N=512, K=288 (in-caps*kernel), C=32 (out-caps), P=16 (pose).
Sharding: out_caps C=32 -> 4 per core across 8 cores (tensor parallel, fully
local math: all reductions are over K/P, and the e-step softmax over N is
per-(k,c) which stays on-core).

Per-core algorithm (layout: n on 128 partitions, free axes (k, c, p)):
  for i in 0..R-1:
    m-step: S0 = sum_k rr, S1 = sum_k rr*v, S2 = sum_k rr*v^2
            means = S1/S0, var = S2/S0 - means^2, std = sqrt(var)
            act = sigmoid(lambda_i * (beta_a - 16*beta_v - S0 * sum_p log(std+eps)))
    e-step (i < R-1):
            pe[n,k,c]  = sum_p (v*is - means*is)^2   (is = 1/std)
            t[n,k,c]   = exp(-0.5*pe + ln(act+eps) - sum_p log(std+eps))
            D[k,c]     = sum_n t  (TensorE ones-matmul, accumulated over n-slabs)
            rr_next    = t / D   (softmax over axis 0 without max-shift;
                                  equal to jax.nn.softmax(zz, axis=0))
"""

import sys
from contextlib import ExitStack

import numpy as np

if "/opt/trn_rl_repo" not in sys.path:
    sys.path.insert(0, "/opt/trn_rl_repo")

import os

INVD_P0 = os.environ.get("KK_INVD_P0", "1") == "1"

# problem dims (hardcoded per harness contract)
N, K, C, P = 512, 288, 32, 16
NCORES = 8
CL = C // NCORES  # out-caps per core
NP = 128          # partitions per slab
EPS = 1e-7


def emit_core_kernel(ctx, tc, outs, ins, R, dims=None):
    """Emit the per-core program. outs=(out_act[N,CL], out_means[N,CL,P]),
    ins=(votes[N,K,CL,P], beta_a[CL], beta_v[CL])."""
    import concourse.mybir as mybir

    n, k, cl, p = dims if dims is not None else (N, K, CL, P)
    nt = n // NP
    f32 = mybir.dt.float32
    AF = mybir.ActivationFunctionType
    AX = mybir.AxisListType
    OP = mybir.AluOpType

    nc = tc.nc
    out_act, out_means = outs
    votes, beta_a, beta_v = ins

    kcl = k * cl
    MMCOL = 384  # f32 cols per PSUM-bank matmul target
    dchunks = [(j, min(MMCOL, kcl - j)) for j in range(0, kcl, MMCOL)]

    # ---- pools ----
    pv = ctx.enter_context(tc.tile_pool(name="pv", bufs=2))       # votes slab
    pc = ctx.enter_context(tc.tile_pool(name="pc", bufs=2))       # [128,k,p/2] scratch
    pscr = ctx.enter_context(tc.tile_pool(name="pscr", bufs=2))   # [128,k] scratch
    pt = ctx.enter_context(tc.tile_pool(name="pt", bufs=1))       # persistent t slabs
    pr = ctx.enter_context(tc.tile_pool(
        name="pr", bufs=int(os.environ.get("KK_RRBUFS", "2"))))  # rr
    pid = ctx.enter_context(tc.tile_pool(name="pid", bufs=1))     # invD_b
    psml = ctx.enter_context(tc.tile_pool(
        name="psml", bufs=int(os.environ.get("KK_SMLBUFS", "2"))))  # small stats
    pone = ctx.enter_context(tc.tile_pool(name="pone", bufs=1))   # constants
    ppsD = ctx.enter_context(tc.tile_pool(name="ppsD", bufs=1, space="PSUM"))
    ppsB = ctx.enter_context(tc.tile_pool(name="ppsB", bufs=1, space="PSUM"))

    # ---- constants ----
    zero_c = pone.tile([NP, 1], f32, tag="zero_c", name="zero_c")
    nc.vector.memset(zero_c[:], 0.0)
    eps_c = pone.tile([NP, 1], f32, tag="eps_c", name="eps_c")
    nc.vector.memset(eps_c[:], EPS)
    nc.const_aps.aps[(f32, 0.0)] = zero_c[:]
    nc.const_aps.aps[(f32, EPS)] = eps_c[:]

    ones_col = pone.tile([NP, 1], f32, tag="ones_col")
    nc.vector.memset(ones_col[:], 1.0)
    ones_row = pone.tile([1, NP], f32, tag="ones_row")
    nc.vector.memset(ones_row[:], 1.0)

    ba_b = pone.tile([NP, cl], f32, tag="ba_b")
    nc.sync.dma_start(out=ba_b[:], in_=beta_a.unsqueeze(0).broadcast_to([NP, cl]))
    bv_b = pone.tile([NP, cl], f32, tag="bv_b")
    nc.sync.dma_start(out=bv_b[:], in_=beta_v.unsqueeze(0).broadcast_to([NP, cl]))
    # ba16bv = beta_a - P*beta_v
    bv16 = pone.tile([NP, cl], f32, tag="bv16")
    nc.scalar.mul(bv16[:], bv_b[:], float(p))
    ba16bv = pone.tile([NP, cl], f32, tag="ba16bv")
    nc.vector.tensor_sub(ba16bv[:], ba_b[:], bv16[:])

    # persistent t slabs (softmax numerators), [128, k, cl] each
    t_sl = [pt.tile([NP, k, cl], f32, tag=f"tsl{s}", name=f"tsl{s}") for s in range(nt)]

    invD_b = None  # [128, k, cl] broadcast of 1/D, refreshed per e-step

    S0_CONST0 = float(k) / float(C)  # sum_k (1/C) for iteration 0

    v_res = {}  # slab idx -> tile for the (up to 2) slabs resident in the
    # double-buffered votes pool; with alternating slab order both survivors
    # are exactly the next phase's first two slabs, saving their DMAs

    for i in range(R):
        lambd = 0.01 * (1.0 - 0.95 ** i)
        is_last = i == R - 1

        if not is_last:
            d_ps = [ppsD.tile([1, sz], f32, tag=f"d{j}", name=f"d{j}") for j, (_, sz) in enumerate(dchunks)]

        def emit_pe(s, si, v_sl, is_, btn, lnw):
            ph = p // 2
            for c in range(cl):
                pe_c = pscr.tile([NP, k], f32, tag="pe_c", name="pe_c")
                for h in range(2):
                    ucb = pc.tile([NP, k, ph], f32, tag="cbuf", name=f"ucb{c}{h}")
                    for pp in range(h * ph, (h + 1) * ph):
                        nc.scalar.activation(
                            out=ucb[:, :, pp - h * ph], in_=v_sl[:, :, c, pp],
                            func=AF.Square,
                            bias=btn[:, c, pp:pp + 1], scale=is_[:, c, pp:pp + 1])
                    if h == 0:
                        nc.vector.tensor_reduce(
                            out=pe_c[:], in_=ucb[:], axis=AX.X, op=OP.add)
                    else:
                        pe_h = pscr.tile([NP, k], f32, tag="pe_h", name="pe_h")
                        nc.vector.tensor_reduce(
                            out=pe_h[:], in_=ucb[:], axis=AX.X, op=OP.add)
                        nc.vector.tensor_add(pe_c[:], pe_c[:], pe_h[:])
                nc.scalar.activation(
                    out=t_sl[s][:, :, c], in_=pe_c[:], func=AF.Exp,
                    bias=lnw[:, c:c + 1], scale=-0.5)
            # partial D accumulation: D[k,c] += sum_n t
            t_flat = t_sl[s][:].rearrange("q a b -> q (a b)")
            for j, (j0, sz) in enumerate(dchunks):
                nc.tensor.matmul(
                    out=d_ps[j][:], lhsT=ones_col[:], rhs=t_flat[:, j0:j0 + sz],
                    start=(si == 0), stop=(si == nt - 1))

        pending = None  # pe work lagged by one slab to overlap with next S-block
        order = list(range(nt)) if i % 2 == 0 else list(reversed(range(nt)))
        for si, s in enumerate(order):
            ns = s * NP
            if s in v_res and i > 0:
                v_sl = v_res[s]  # still resident from previous phase
            else:
                v_sl = pv.tile([NP, k, cl, p], f32, tag="vslab", name="vslab")
                nc.sync.dma_start(out=v_sl[:], in_=votes[ns:ns + NP])
            v_res = {s: v_sl, **{kk_: vv for kk_, vv in v_res.items() if kk_ != s}}
            v_res = dict(list(v_res.items())[:2])  # keep the 2 newest

            S1 = psml.tile([NP, cl, p], f32, tag="S1")
            S2 = psml.tile([NP, cl, p], f32, tag="S2")

            ph = p // 2

            def se_red(src_slice, accum_slot, func=AF.Copy):
                """k-sum on ScalarE: accum_out = sum_k func(src); out -> junk."""
                junk = pscr.tile([NP, k], f32, tag="junk", name="junk")
                nc.scalar.activation(out=junk[:], in_=src_slice, func=func,
                                     scale=1.0, accum_out=accum_slot)

            if i == 0:
                # uniform rr = 1/C: raw sums over k
                nc.vector.tensor_reduce(
                    out=S1[:], in_=v_sl[:].transpose([0, 2, 3, 1]), axis=AX.X, op=OP.add)
                # S2 = sum_k v^2: half the (c,h) groups fused on SE
                # (Square+accum per pose), half as SE-square + VE group reduce
                n_s20 = int(os.environ.get("KK_NS20", "4"))
                for c in range(cl):
                    for h in range(2):
                        sl = slice(h * ph, (h + 1) * ph)
                        if (c * 2 + h) % 2 < (n_s20 / 4.0):
                            for pp in range(h * ph, (h + 1) * ph):
                                se_red(v_sl[:, :, c, pp], S2[:, c, pp:pp + 1],
                                       func=AF.Square)
                        else:
                            sq_c = pc.tile([NP, k, ph], f32, tag="cbuf", name="sqc")
                            nc.scalar.square(sq_c[:], v_sl[:, :, c, sl])
                            nc.vector.tensor_reduce(
                                out=S2[:, c, sl],
                                in_=sq_c[:].transpose([0, 2, 1]), axis=AX.X, op=OP.add)
            else:
                rr = pr.tile([NP, k, cl], f32, tag="rr")
                nc.vector.tensor_mul(rr[:], t_sl[s][:], invD_b[:])
                S0 = psml.tile([NP, cl], f32, tag="S0")
                nc.vector.tensor_reduce(
                    out=S0[:], in_=rr[:].transpose([0, 2, 1]), axis=AX.X, op=OP.add)
                # products on VE; k-reduces split between SE activation-accum
                # and VE group reduces, h-pairs interleaved so VE's in-place
                # second multiply overlaps SE's reads of the sibling group.
                # For n_q groups, S2 = sum_k (sqrt(rr)*v)^2 via SE Square-accum
                # on q = v*sqrt(rr) — q is independent of the prod->S1 chain.
                n_s1_se = int(os.environ.get("KK_NS1", "8"))
                n_s2_se = int(os.environ.get("KK_NS2", "0"))
                n_q = int(os.environ.get("KK_NQ", "0"))
                srr = None
                if n_q > 0:
                    srr = pr.tile([NP, k, cl], f32, tag="srr", name="srr", bufs=1)
                    nc.scalar.sqrt(srr[:], rr[:])
                for c in range(cl):
                    prods = []
                    for h in range(2):
                        sl = slice(h * ph, (h + 1) * ph)
                        prod_c = pc.tile([NP, k, ph], f32, tag="cbuf", name="prodc")
                        vs = v_sl[:, :, c, sl]
                        rr_bc = rr[:, :, c].unsqueeze(-1).broadcast_to([NP, k, ph])
                        nc.vector.tensor_mul(prod_c[:], vs, rr_bc)
                        prods.append((prod_c, vs, sl, h))
                    for prod_c, vs, sl, h in prods:
                        if (c * 2 + h) < n_s1_se:
                            for jj in range(ph):
                                se_red(prod_c[:, :, jj],
                                       S1[:, c, h * ph + jj:h * ph + jj + 1])
                        else:
                            nc.vector.tensor_reduce(
                                out=S1[:, c, sl],
                                in_=prod_c[:].transpose([0, 2, 1]), axis=AX.X, op=OP.add)
                    for prod_c, vs, sl, h in prods:
                        if (c * 2 + h) < n_q:
                            q_c = pc.tile([NP, k, ph], f32, tag="cbuf", name="qc")
                            srr_bc = srr[:, :, c].unsqueeze(-1).broadcast_to([NP, k, ph])
                            nc.vector.tensor_mul(q_c[:], vs, srr_bc)
                            for jj in range(ph):
                                se_red(q_c[:, :, jj],
                                       S2[:, c, h * ph + jj:h * ph + jj + 1],
                                       func=AF.Square)
                        else:
                            if os.environ.get("KK_GP", "0") == "1":
                                nc.gpsimd.tensor_mul(prod_c[:], prod_c[:], vs)
                            else:
                                nc.vector.tensor_mul(prod_c[:], prod_c[:], vs)
                            if (c * 2 + h) < n_s2_se:
                                for jj in range(ph):
                                    se_red(prod_c[:, :, jj],
                                           S2[:, c, h * ph + jj:h * ph + jj + 1])
                            else:
                                nc.vector.tensor_reduce(
                                    out=S2[:, c, sl],
                                    in_=prod_c[:].transpose([0, 2, 1]), axis=AX.X, op=OP.add)

            # ---- m-step finalize ----
            means = psml.tile([NP, cl, p], f32, tag="means")
            ex2 = psml.tile([NP, cl, p], f32, tag="ex2")
            if i == 0:
                nc.scalar.mul(means[:], S1[:], 1.0 / float(k))
                nc.scalar.mul(ex2[:], S2[:], 1.0 / float(k))
            else:
                invS0 = psml.tile([NP, cl], f32, tag="invS0")
                nc.vector.reciprocal(invS0[:], S0[:])
                is0b = invS0[:].unsqueeze(-1).broadcast_to([NP, cl, p])
                nc.vector.tensor_mul(means[:], S1[:], is0b)
                nc.vector.tensor_mul(ex2[:], S2[:], is0b)
            msq = psml.tile([NP, cl, p], f32, tag="msq")
            nc.scalar.square(msq[:], means[:])
            var = psml.tile([NP, cl, p], f32, tag="var")
            nc.vector.tensor_sub(var[:], ex2[:], msq[:])
            std = psml.tile([NP, cl, p], f32, tag="std")
            nc.scalar.sqrt(std[:], var[:])
            logstd = psml.tile([NP, cl, p], f32, tag="logstd")
            nc.scalar.activation(logstd[:], std[:], AF.Ln, bias=EPS, scale=1.0)
            sumlog = psml.tile([NP, cl], f32, tag="sumlog")
            nc.vector.tensor_reduce(out=sumlog[:], in_=logstd[:], axis=AX.X, op=OP.add)

            cterm = psml.tile([NP, cl], f32, tag="cterm")
            if i == 0:
                nc.scalar.mul(cterm[:], sumlog[:], S0_CONST0)
            else:
                nc.vector.tensor_mul(cterm[:], S0[:], sumlog[:])
            actin = psml.tile([NP, cl], f32, tag="actin")
            nc.vector.tensor_sub(actin[:], ba16bv[:], cterm[:])
            act = psml.tile([NP, cl], f32, tag="act")
            nc.scalar.activation(act[:], actin[:], AF.Sigmoid, bias=0.0, scale=float(lambd))

            if is_last:
                nc.sync.dma_start(out=out_act[ns:ns + NP], in_=act[:])
                nc.sync.dma_start(out=out_means[ns:ns + NP], in_=means[:])
                continue

            # ---- e-step ----
            is_ = psml.tile([NP, cl, p], f32, tag="is_")
            nc.vector.reciprocal(is_[:], std[:])
            mis = psml.tile([NP, cl, p], f32, tag="mis")
            nc.vector.tensor_mul(mis[:], means[:], is_[:])
            btn = psml.tile([NP, cl, p], f32, tag="btn")
            nc.scalar.mul(btn[:], mis[:], -1.0)
            lnact = psml.tile([NP, cl], f32, tag="lnact")
            nc.scalar.activation(lnact[:], act[:], AF.Ln, bias=EPS, scale=1.0)
            lnw = psml.tile([NP, cl], f32, tag="lnw")
            nc.vector.tensor_sub(lnw[:], lnact[:], sumlog[:])

            emit_pe(s, si, v_sl, is_, btn, lnw)

        if is_last:
            continue

        # ---- finalize D -> invD broadcast across partitions ----
        # reciprocal lands in partition 0 of invD_b, then a [1,128]-ones
        # matmul replicates it to all partitions (overwriting row 0 equally)
        invD_b = pid.tile([NP, k, cl], f32, tag="invD_b")
        invDb_flat = invD_b[:].rearrange("q a b -> q (a b)")
        if INVD_P0:
            row = invDb_flat[0:1, :]
        else:
            invD_row = pone.tile([1, kcl], f32, tag="invD_row", name="invD_row")
            row = invD_row[:]
        for j, (j0, sz) in enumerate(dchunks):
            nc.vector.reciprocal(row[:, j0:j0 + sz], d_ps[j][:])
        for j, (j0, sz) in enumerate(dchunks):
            b_ps = ppsB.tile([NP, MMCOL], f32, tag=f"b{j}", name=f"b{j}")
            nc.tensor.matmul(
                out=b_ps[:, :sz], lhsT=ones_row[:], rhs=row[:, j0:j0 + sz],
                start=True, stop=True)
            nc.vector.tensor_copy(out=invDb_flat[:, j0:j0 + sz], in_=b_ps[:, :sz])


def build_program(R, dims=None):
    import concourse.bacc as bacc
    import concourse.mybir as mybir
    import concourse.tile as tile

    n, k, cl, p = dims if dims is not None else (N, K, CL, P)
    f32 = mybir.dt.float32

    nc = bacc.Bacc("TRN2", target_bir_lowering=False, debug=False,
                   enable_asserts=False, num_devices=NCORES)
    votes = nc.dram_tensor("votes", [n, k, cl, p], f32, kind="ExternalInput")
    beta_a = nc.dram_tensor("beta_a", [cl], f32, kind="ExternalInput")
    beta_v = nc.dram_tensor("beta_v", [cl], f32, kind="ExternalInput")
    out_act = nc.dram_tensor("out_act", [n, cl], f32, kind="ExternalOutput")
    out_means = nc.dram_tensor("out_means", [n, cl, p], f32, kind="ExternalOutput")

    with tile.TileContext(nc) as tc:
        with ExitStack() as ctx:
            emit_core_kernel(
                ctx, tc,
                (out_act.ap(), out_means.ap()),
                (votes.ap(), beta_a.ap(), beta_v.ap()),
                R, dims=dims)
    nc.compile()
    return nc


_CACHE = {}


def _get_program(R):
    if R not in _CACHE:
        _CACHE[R] = build_program(R)
    return _CACHE[R]


_RUNNER = {}


def _get_runner(R):
    """Build the sharded PJRT executable once per R (mirrors
    bass2jax.run_bass_via_pjrt's multi-core path, cached across calls)."""
    if R in _RUNNER:
        return _RUNNER[R]
    import jax
    from jax.sharding import Mesh, PartitionSpec
    from jax.experimental.shard_map import shard_map
    import concourse.mybir as mybir
    from concourse import bass2jax

    nc = _get_program(R)
    bass2jax.install_neuronx_cc_hook()

    partition_name = nc.partition_id_tensor.name if nc.partition_id_tensor else None
    in_names, out_names, out_avals, zero_shapes = [], [], [], []
    for alloc in nc.m.functions[0].allocations:
        if not isinstance(alloc, mybir.MemoryLocationSet):
            continue
        name = alloc.memorylocations[0].name
        if alloc.kind == "ExternalInput":
            if name != partition_name:
                in_names.append(name)
        elif alloc.kind == "ExternalOutput":
            out_names.append(name)
            shape = tuple(alloc.tensor_shape)
            dtype = mybir.dt.np(alloc.dtype)
            out_avals.append(jax.core.ShapedArray(shape, dtype))
            zero_shapes.append((shape, dtype))
    n_params = len(in_names)
    n_outs = len(out_avals)
    all_in_names = list(in_names) + list(out_names)
    if partition_name is not None:
        all_in_names.append(partition_name)
    donate = tuple(range(n_params, n_params + n_outs))

    def _body(*args):
        operands = list(args)
        if partition_name is not None:
            operands.append(bass2jax.partition_id_tensor())
        outs = bass2jax._bass_exec_p.bind(
            *operands,
            out_avals=tuple(out_avals),
            in_names=tuple(all_in_names),
            out_names=tuple(out_names),
            lowering_input_output_aliases=(),
            sim_require_finite=True,
            sim_require_nnan=True,
            nc=nc,
        )
        return tuple(outs)

    devices = jax.devices()[:NCORES]
    mesh = Mesh(np.asarray(devices), ("core",))
    in_specs = (PartitionSpec("core"),) * (n_params + n_outs)
    out_specs = (PartitionSpec("core"),) * n_outs
    sharded = jax.jit(
        shard_map(_body, mesh=mesh, in_specs=in_specs, out_specs=out_specs,
                  check_rep=False),
        donate_argnums=donate, keep_unused=True)

    _RUNNER[R] = (sharded, in_names, out_names, out_avals, zero_shapes)
    return _RUNNER[R]


def kernel(**inputs):
    votes = np.asarray(inputs["votes"], dtype=np.float32)
    beta_a = np.asarray(inputs["beta_a"], dtype=np.float32).reshape(-1)
    beta_v = np.asarray(inputs["beta_v"], dtype=np.float32).reshape(-1)
    R = int(inputs.get("routings", 3))
    assert votes.shape == (N, K, C, P)

    in_maps = []
    for j in range(NCORES):
        cs = j * CL
        in_maps.append({
            "votes": np.ascontiguousarray(votes[:, :, cs:cs + CL, :]),
            "beta_a": np.ascontiguousarray(beta_a[cs:cs + CL]),
            "beta_v": np.ascontiguousarray(beta_v[cs:cs + CL]),
        })

    sharded, in_names, out_names, out_avals, zero_shapes = _get_runner(R)
    concat_in = [
        np.concatenate([np.asarray(m[name]) for m in in_maps], axis=0)
        for name in in_names
    ]
    concat_zeros = [
        np.zeros((NCORES * s[0], *s[1:]), d) for s, d in zero_shapes
    ]
    out_arrs = sharded(*concat_in, *concat_zeros)
    res = {
        name: np.asarray(out_arrs[i]).reshape(NCORES, *out_avals[i].shape)
        for i, name in enumerate(out_names)
    }
    out_act = np.concatenate(
        [res["out_act"][c].reshape(N, 1, CL, 1) for c in range(NCORES)], axis=2)
    out_means = np.concatenate(
        [res["out_means"][c].reshape(N, 1, CL, P) for c in range(NCORES)], axis=2)
    return out_act, out_means
